# revision 66
# baseline (speedup 1.0000x reference)
"""Trainium2 Bass kernel for nn_MicroExpert (sparse_attention).

Reference model (B=2, T=2048, D=512, H=8, HD=64):
  v_in = conv1d(x, k=3, pad=1); MHA(q=x, k=x, v=v_in) with banded mask
  |i-j| <= 256; h = LN(x + attn); out = LN(h + FFN(h)).

Sharding: data-parallel over (batch, 512-token chunk) -> 8 independent
cores, no collectives.  Each core recomputes the K/V halo (+-256 tokens,
zero-padded at sequence edges); pad keys drop out of the softmax exactly
because the valid-kv indicator (`vones`) forms the ones-columns of v.

Key structure:
- K/Q/V projections run in fp8 (e4m3, weights x16-prescaled) with
  DoubleRow so each matmul contracts 256 rows; the conv is folded into
  the V projection on the host: v[t] = sum_d U_d x[t+d-1],
  U_d = Wv conv_w[:,:,d].
- Attention keeps a [feature, token] layout.  Scores for a head pair
  are issued into one PSUM tile with alternating 64-row groups (both
  K=64 matmuls run concurrently) and exponentiated by a single Exp; the
  scalar engine runs only Exp during attention so its activation table
  never swaps.  The v stationary carries 64 replicated valid-kv ones
  columns, so the ctx matmul emits the softmax denominator on PSUM rows
  64-127, partition-aligned with ctx - normalization is one reciprocal
  plus two multiplies, no transpose/broadcast.  The qt loop is software-
  pipelined one iteration deep to hide the denominator latency.
- LayerNorm uses bn_stats/bn_aggr; LN1 finishes after the qt loop.
- FFN2 uses m1 as the stationary operand so its output lands directly
  in [token, feature]; residual2 + LN2 + store pipeline per query tile.
"""

import os
import sys

import numpy as np

sys.path.insert(0, "/opt/trn_rl_repo")

import concourse.bass as bass
import concourse.mybir as mybir
import concourse.tile as tile
from concourse import bacc
from concourse.bass_utils import run_bass_kernel_spmd

BF16 = mybir.dt.bfloat16
F32 = mybir.dt.float32

B, T, D, H, HD = 2, 2048, 512, 8, 64
S = 512          # tokens per core
KV = 1024        # extended kv tokens per core (S + 2*256)
XE = 1026        # x_ext width (KV + 2 for conv halo)
NQT = 4          # 128-query tiles per core
NKT = 5          # relative 128-key tiles per query tile
F = 1024         # FFN hidden
EPS = 1e-5
N_CORES = 8

_cached = {}


def _build_program():
    nc = bacc.Bacc("TRN2", target_bir_lowering=False, debug=False)

    # ---- DRAM tensors -------------------------------------------------
    def din(name, shape, dt):
        return nc.dram_tensor(name, shape, dt, kind="ExternalInput").ap()

    F8 = mybir.dt.float8e4
    XE8 = 1040  # fp8 xt block stride (DoubleRow needs step % 16 == 0)

    # all inputs are pre-layouted [128, N] SBUF images (host does the packing)
    xt_d = din("xt", [128, 4 * XE], BF16)
    xt8_d = din("xt8", [128, 4 * XE8], F8)
    wk_d = din("wk", [128, 2048], F8)                 # x16 scaled, fp8
    wq_d = din("wq", [128, 2048], F8)                 # x16 scaled, fp8
    wu_d = din("wu", [128, 6144], F8)                 # x16 scaled, fp8
    wo_d = din("wo", [128, 2048], BF16)
    w1_d = din("w1", [128, 4096], BF16)
    w2t_d = din("w2t", [128, 4096], BF16)
    mask_d = din("mask01", [128, 256], BF16)          # [tril | triu] 0/1
    vones_d = din("vones", [128, 512], BF16)          # valid-kv, x64 replicated
    ident_d = din("ident", [128, 128], BF16)

    out_d = nc.dram_tensor("out", [128, 2048], BF16, kind="ExternalOutput").ap()

    with tile.TileContext(nc) as tc:
        from contextlib import ExitStack

        with ExitStack() as ctx:
            const = ctx.enter_context(tc.tile_pool(name="const", bufs=1))

            # ---- load constants/weights (consumption order, parallel queues)
            def load_w(dram, cols, dt=BF16, parts=128, nchunk=1, eng=None):
                t = const.tile([parts, cols], dt, name=f"w_{dram.tensor.name}")
                step = cols // nchunk
                e = eng if eng is not None else nc.sync
                for c0 in range(0, cols, step):
                    e.dma_start(t[:, c0:c0 + step], dram[:, c0:c0 + step])
                return t

            # single queue, strict consumption order (priority = order)
            wk_sb = load_w(wk_d, 2048, dt=F8)
            xt8_sb = load_w(xt8_d, 4 * XE8, dt=F8, nchunk=4)
            wq_sb = load_w(wq_d, 2048, dt=F8)
            xt_sb = load_w(xt_d, 4 * XE, nchunk=2)
            wu_sb = load_w(wu_d, 6144, dt=F8)
            wo_sb = load_w(wo_d, 2048)
            w1_sb = load_w(w1_d, 4096)
            w2t_sb = load_w(w2t_d, 4096)
            mask_sb = load_w(mask_d, 256)
            vones_sb = load_w(vones_d, 512)
            ident_sb = load_w(ident_d, 128)

            # persistent activations
            kt_sb = const.tile([128, 4 * KV], BF16)    # [oc-block][kv]
            q_sb = const.tile([128, 4 * S], BF16)      # [oc-block][tok]
            v_sb = const.tile([128, 8 * 1024], BF16)   # [kvtile][(v_h | ones x64) x 8]
            ctx_sb = const.tile([128, 4 * 512], BF16)  # [qt-block][pair-block][q]
            hn_sb = const.tile([128, 4 * 512], BF16)   # [qt-block][D]  (LN1 out, [tok,D])
            h1t_sb = const.tile([128, 4 * 512], BF16)  # [dc-block][tok]
            m1_sb = const.tile([128, 8 * 512], BF16)   # [fc-block][tok]
            out_sb = const.tile([128, 4 * 512], BF16)  # [qt-block][D]
            stats = const.tile([128, 32], F32)
            eps_sb = const.tile([128, 1], F32)
            nc.gpsimd.memset(eps_sb[:], float(EPS))

            xt_v = xt_sb[:].rearrange("p (c w) -> p c w", c=4)
            # valid-kv indicator, replicated across 64 stationary columns of
            # each head's v slice: the ctx matmul then emits the softmax
            # denominator (pad-kv excluded) on PSUM rows 64-127, partition-
            # aligned with the ctx rows, so normalization needs no
            # transpose/broadcast of the denominator at all.
            v_thw = v_sb[:].rearrange("p (t h w) -> p t h w", t=8, h=8)
            for h in range(8):
                nc.gpsimd.tensor_copy(
                    v_thw[:, :, h, 64:128],
                    vones_sb[:].rearrange("p (t o) -> p t o", t=8),
                )

            # ---- projections: kT, qT, v (fp8 DoubleRow) ----------------
            with tc.tile_pool(name="pp", bufs=2, space="PSUM") as pp_pool:
                xt8_v = xt8_sb[:].rearrange("p (c w) -> p c w", c=4)
                # [p, j(kc-pair), k(kc-in-pair), oc*128]
                wkv = wk_sb[:].rearrange("p (j k w) -> p j k w", j=2, k=2)
                wqv = wq_sb[:].rearrange("p (j k w) -> p j k w", j=2, k=2)
                for oc in range(4):
                    for half in range(2):
                        pp = pp_pool.tile([128, 512], F32, tag="pp")
                        for j in range(2):
                            nc.tensor.matmul(
                                pp[:],
                                wkv[:, j, :, 128 * oc: 128 * oc + 128],
                                xt8_v[:, 2 * j:2 * j + 2,
                                      1 + 512 * half: 1 + 512 * half + 512],
                                start=(j == 0), stop=(j == 1),
                                perf_mode=mybir.MatmulPerfMode.DoubleRow,
                            )
                        dst = kt_sb[:, KV * oc + 512 * half: KV * oc + 512 * half + 512]
                        if (oc + half) % 2 == 0:
                            nc.scalar.copy(dst, pp[:])
                        else:
                            nc.vector.tensor_copy(dst, pp[:])
                for oc in range(4):
                    pp = pp_pool.tile([128, 512], F32, tag="pp")
                    for j in range(2):
                        nc.tensor.matmul(
                            pp[:],
                            wqv[:, j, :, 128 * oc: 128 * oc + 128],
                            xt8_v[:, 2 * j:2 * j + 2, 257: 257 + 512],
                            start=(j == 0), stop=(j == 1),
                            perf_mode=mybir.MatmulPerfMode.DoubleRow,
                        )
                    nc.vector.tensor_copy(q_sb[:, 512 * oc: 512 * oc + 512], pp[:])
                xt8_v = xt8_sb[:].rearrange("p (c w) -> p c w", c=4)
                wu_v = wu_sb[:].rearrange("p (j b w) -> p j b w", j=6, b=2)
                for tt in range(8):
                    pp = pp_pool.tile([128, 512], F32, tag="pp")
                    for j in range(6):  # DoubleRow: two (tap,dc) blocks per MM
                        tap, k2 = j // 2, 2 * (j % 2)
                        nc.tensor.matmul(
                            pp[:],
                            xt8_v[:, k2:k2 + 2, 128 * tt + tap: 128 * tt + tap + 128],
                            wu_v[:, j],
                            start=(j == 0), stop=(j == 5),
                            perf_mode=mybir.MatmulPerfMode.DoubleRow,
                        )
                    vv = v_sb[:, 1024 * tt: 1024 * tt + 1024].rearrange(
                        "p (h w) -> p h w", h=8
                    )
                    src = pp[:].rearrange("p (h w) -> p h w", h=8)
                    if tt % 2 == 0:
                        nc.scalar.activation(
                            vv[:, :, 0:64], src,
                            mybir.ActivationFunctionType.Copy, scale=1.0 / 16,
                        )
                    else:
                        nc.vector.tensor_scalar_mul(vv[:, :, 0:64], src, 1.0 / 16)

            # ---- attention (software-pipelined over qt) ----------------
            with ExitStack() as actx:
                sc_pool = actx.enter_context(tc.tile_pool(name="scps", bufs=2, space="PSUM"))
                cx_pool = actx.enter_context(tc.tile_pool(name="cxps", bufs=1, space="PSUM"))
                ex_pool = actx.enter_context(tc.tile_pool(name="expsb", bufs=4))
                dn_pool = actx.enter_context(tc.tile_pool(name="densb", bufs=2))
                r1t_pool = actx.enter_context(tc.tile_pool(name="r1t", bufs=2))
                r1_pool = actx.enter_context(tc.tile_pool(name="r1", bufs=4))

                cxps_t = [None] * NQT

                def sc_stage(qt, p):
                    """Scores + exp + mask for head pair p (heads 2p, 2p+1):
                    one PSUM tile and a single Exp for both halves."""
                    scps = sc_pool.tile([128, 1280], F32, tag="sc")
                    # interleave halves so row-groups 0-63 / 64-127 overlap on PE
                    for kt in range(NKT):
                        for half in range(2):
                            hp = 64 * half
                            kcol = KV * p + 128 * (qt + kt)
                            nc.tensor.matmul(
                                scps[:, 640 * half + 128 * kt: 640 * half + 128 * kt + 128],
                                kt_sb[hp:hp + 64, kcol:kcol + 128],
                                q_sb[hp:hp + 64, 512 * p + 128 * qt: 512 * p + 128 * qt + 128],
                                start=True, stop=True,
                            )
                    expair = ex_pool.tile([128, 1280], BF16, tag="ex")
                    nc.scalar.activation(
                        expair[:], scps[:], mybir.ActivationFunctionType.Exp,
                        scale=float(1.0 / (np.sqrt(HD) * 256.0)),
                    )
                    exs = []
                    for half in range(2):
                        ex = expair[:, 640 * half: 640 * half + 640]
                        # band mask on relative tiles 0 and 4 (multiplicative 0/1)
                        ex_edge = ex.rearrange("p (a b) -> p a b", a=5)[:, ::4, :]
                        nc.vector.tensor_mul(
                            ex_edge, ex_edge,
                            mask_sb[:].rearrange("p (n w) -> p n w", n=2),
                        )
                        exs.append(ex)
                    return exs

                def ctx_stage(qt, p, exs):
                    if cxps_t[qt] is None:
                        cxps = cx_pool.tile([128, 1024], F32, tag="cx", name=f"cx{qt}")
                        cxps_t[qt] = cxps
                    cxps = cxps_t[qt]
                    for half in range(2):
                        h = 2 * p + half
                        ex = exs[half]
                        for kt in range(NKT):
                            nc.tensor.matmul(
                                cxps[:, 128 * h: 128 * h + 128],
                                v_sb[:, 1024 * (qt + kt) + 128 * h: 1024 * (qt + kt) + 128 * h + 128],
                                ex[:, 128 * kt: 128 * kt + 128],
                                start=(kt == 0), stop=(kt == NKT - 1),
                                skip_group_check=True,
                            )

                def den_stage(qt):
                    """Copy ctx and replicated denominator out of PSUM; take
                    1/den at partition base 0 (reciprocal_approx_* misreads
                    PSUM and non-zero partition bases on hardware)."""
                    cxps = cxps_t[qt]
                    ctxu = dn_pool.tile([64, 1024], BF16, tag="ctxu")
                    nc.vector.tensor_copy(ctxu[:], cxps[0:64, :])
                    den = dn_pool.tile([64, 1024], F32, tag="den")
                    nc.vector.tensor_copy(den[:], cxps[64:128, :])
                    rden = dn_pool.tile([64, 1024], F32, tag="rden")
                    nc.vector.reciprocal_approx_fast(rden[:], den[:])
                    cxps_t[qt] = None
                    return ctxu, rden

                def bcast_norm(qt, ctxu, rden):
                    cxv = ctxu[:].rearrange("p (a b) -> p a b", a=8)
                    bcv = rden[:].rearrange("p (a b) -> p a b", a=8)
                    ctxv = ctx_sb[:].rearrange("p (a b) -> p a b", a=16)
                    for par in range(2):
                        nc.vector.tensor_mul(
                            ctxv[64 * par:64 * par + 64, 4 * qt:4 * qt + 4, :],
                            cxv[:, par::2, :],
                            bcv[:, par::2, :],
                        )

                r1_tiles = []

                def oproj_r1(qt):
                    """out_proj + residual + transpose to [tok, feat]."""
                    atps = sc_pool.tile([128, 512], F32, tag="sc")
                    for oc in range(4):
                        for pc in range(4):
                            nc.tensor.matmul(
                                atps[:, 128 * oc: 128 * oc + 128],
                                wo_sb[:, 128 * (4 * pc + oc): 128 * (4 * pc + oc) + 128],
                                ctx_sb[:, 512 * qt + 128 * pc: 512 * qt + 128 * pc + 128],
                                start=(pc == 0), stop=(pc == 3),
                            )
                    r1t = r1t_pool.tile([128, 512], BF16, tag="r1t")
                    nc.vector.tensor_add(
                        r1t[:], atps[:], xt_v[:, :, 257 + 128 * qt: 257 + 128 * qt + 128]
                    )
                    tpps = sc_pool.tile([128, 512], BF16, tag="sc")
                    for dc in range(4):
                        nc.tensor.transpose(
                            tpps[:, 128 * dc: 128 * dc + 128],
                            r1t[:, 128 * dc: 128 * dc + 128],
                            ident_sb[:],
                        )
                    r1 = r1_pool.tile([128, 512], BF16, tag="r1", name=f"r1_{qt}")
                    nc.vector.tensor_copy(r1[:], tpps[:])
                    r1_tiles.append(r1)
                    b6 = dn_pool.tile([128, 6], F32, tag="b6")
                    nc.vector.bn_stats(b6[:], r1[:])
                    nc.vector.bn_aggr(stats[:, 2 * qt:2 * qt + 2], b6[:])

                # software pipeline with one-iteration lookahead: qt's
                # scores/ctx cover qt-1's denominator-chain latency, then
                # qt-1's normalize/out_proj fill PE behind them.  LayerNorm1
                # is deferred past the loop so the scalar engine runs Exp
                # only (no activation-table swaps) and no PE op waits on
                # the LN chain.
                pend = None
                for qt in range(NQT):
                    ex0 = sc_stage(qt, 0)
                    ex1 = sc_stage(qt, 1)
                    ctx_stage(qt, 0, ex0)
                    ex2 = sc_stage(qt, 2)
                    ctx_stage(qt, 1, ex1)
                    ex3 = sc_stage(qt, 3)
                    ctx_stage(qt, 2, ex2)
                    if pend is not None:
                        bcast_norm(qt - 1, *pend)
                    ctx_stage(qt, 3, ex3)
                    if pend is not None:
                        oproj_r1(qt - 1)
                    pend = den_stage(qt)
                def ln1_finish(qt):
                    nc.scalar.activation(
                        stats[:, 16 + qt:17 + qt], stats[:, 2 * qt + 1:2 * qt + 2],
                        mybir.ActivationFunctionType.Sqrt, bias=eps_sb[:, 0:1],
                    )
                    nc.vector.reciprocal(
                        stats[:, 20 + qt:21 + qt], stats[:, 16 + qt:17 + qt]
                    )
                    nc.vector.tensor_scalar(
                        hn_sb[:, 512 * qt: 512 * qt + 512], r1_tiles[qt][:],
                        stats[:, 2 * qt:2 * qt + 1], stats[:, 20 + qt:21 + qt],
                        op0=mybir.AluOpType.subtract, op1=mybir.AluOpType.mult,
                    )
                    tpps2 = sc_pool.tile([128, 512], BF16, tag="sc")
                    for dc in range(4):
                        nc.tensor.transpose(
                            tpps2[:, 128 * dc: 128 * dc + 128],
                            hn_sb[:, 512 * qt + 128 * dc: 512 * qt + 128 * dc + 128],
                            ident_sb[:],
                        )
                    h1tv = h1t_sb[:].rearrange("p (c w) -> p c w", c=4)
                    nc.vector.tensor_copy(
                        h1tv[:, :, 128 * qt: 128 * qt + 128], tpps2[:]
                    )

                bcast_norm(NQT - 1, *pend)
                oproj_r1(NQT - 1)
                for qt in range(NQT):
                    ln1_finish(qt)

            # ---- FFN1 --------------------------------------------------
            with tc.tile_pool(name="fps", bufs=2, space="PSUM") as f_pool:
                for fc in range(8):
                    fps = f_pool.tile([128, 512], F32, tag="f")
                    for dc in range(4):
                        nc.tensor.matmul(
                            fps[:],
                            w1_sb[:, 128 * (8 * dc + fc): 128 * (8 * dc + fc) + 128],
                            h1t_sb[:, 512 * dc: 512 * dc + 512],
                            start=(dc == 0), stop=(dc == 3),
                        )
                    nc.vector.tensor_scalar(
                        m1_sb[:, 512 * fc: 512 * fc + 512], fps[:],
                        0.0, 0.0,
                        op0=mybir.AluOpType.add, op1=mybir.AluOpType.max,
                    )

                # ---- FFN2 ([tok, feat] direct) + residual2 + LN2 + store
                with ExitStack() as fctx:
                    r2_pool = fctx.enter_context(tc.tile_pool(name="r2", bufs=2))
                    sq2_pool = fctx.enter_context(tc.tile_pool(name="sq2", bufs=2))
                    for qt in range(NQT):
                        fps = f_pool.tile([128, 512], F32, tag="f")
                        for fc in range(8):
                            nc.tensor.matmul(
                                fps[:],
                                m1_sb[:, 512 * fc + 128 * qt: 512 * fc + 128 * qt + 128],
                                w2t_sb[:, 512 * fc: 512 * fc + 512],
                                start=(fc == 0), stop=(fc == 7),
                            )
                        r2 = r2_pool.tile([128, 512], BF16, tag="r2")
                        nc.vector.tensor_add(
                            r2[:], fps[:], hn_sb[:, 512 * qt: 512 * qt + 512]
                        )
                        b6 = sq2_pool.tile([128, 6], F32, tag="b6f")
                        mv = stats[:, 8 + 2 * qt:10 + 2 * qt]
                        nc.vector.bn_stats(b6[:], r2[:])
                        nc.vector.bn_aggr(mv, b6[:])
                        sd = stats[:, 24 + qt:25 + qt]
                        nc.scalar.activation(
                            sd, stats[:, 9 + 2 * qt:10 + 2 * qt],
                            mybir.ActivationFunctionType.Sqrt, bias=eps_sb[:, 0:1],
                        )
                        rstd = stats[:, 28 + qt:29 + qt]
                        nc.vector.reciprocal(rstd, sd)
                        nc.vector.tensor_scalar(
                            out_sb[:, 512 * qt: 512 * qt + 512], r2[:],
                            stats[:, 8 + 2 * qt:9 + 2 * qt], rstd,
                            op0=mybir.AluOpType.subtract, op1=mybir.AluOpType.mult,
                        )
                        nc.sync.dma_start(
                            out_d[:, 512 * qt: 512 * qt + 512],
                            out_sb[:, 512 * qt: 512 * qt + 512],
                        )

    nc.compile()
    return nc


def _prep_host(inputs):
    x = np.asarray(inputs["x"], np.float32)
    conv_w = np.asarray(inputs["conv_w"], np.float32)
    conv_b = np.asarray(inputs["conv_b"], np.float32)
    in_w = np.asarray(inputs["in_proj_w"], np.float32)
    in_b = np.asarray(inputs["in_proj_b"], np.float32)
    out_w = np.asarray(inputs["out_proj_w"], np.float32)
    out_b = np.asarray(inputs["out_proj_b"], np.float32)
    w1 = np.asarray(inputs["w1"], np.float32)
    b1 = np.asarray(inputs["b1"], np.float32)
    w2 = np.asarray(inputs["w2"], np.float32)
    b2 = np.asarray(inputs["b2"], np.float32)
    g1 = np.asarray(inputs["ln1_g"], np.float32)
    bb1 = np.asarray(inputs["ln1_b"], np.float32)
    g2 = np.asarray(inputs["ln2_g"], np.float32)
    bb2 = np.asarray(inputs["ln2_b"], np.float32)

    for nm, v in (("conv_b", conv_b), ("in_proj_b", in_b), ("out_proj_b", out_b),
                  ("b1", b1), ("b2", b2)):
        if np.any(v != 0):
            raise NotImplementedError(f"nonzero {nm} unsupported")
    if np.any(g1 != 1) or np.any(bb1 != 0) or np.any(g2 != 1) or np.any(bb2 != 0):
        raise NotImplementedError("nontrivial layernorm affine unsupported")

    Wq, Wk, Wv = in_w[:D], in_w[D:2 * D], in_w[2 * D:]
    U = [(Wv @ conv_w[:, :, d]) for d in range(3)]  # v[t] = sum U_d @ x[t+d-1]

    def img(stack):  # [n, 128, w] slices -> [128, n*w] SBUF image
        a = np.asarray(stack, np.float32)
        return np.ascontiguousarray(a.transpose(1, 0, 2).reshape(128, -1))

    def slc16(W):  # W used as out = W @ x  -> lhsT slices of W.T
        WT = np.ascontiguousarray(W.T)
        return img([
            WT[128 * kc:128 * kc + 128, 128 * oc:128 * oc + 128]
            for kc in range(4) for oc in range(4)
        ])

    wk_a = slc16(Wk)
    wq_a = slc16(Wq)
    wo_a = slc16(out_w)
    wu_a = img([
        np.ascontiguousarray(U[tap].T)[128 * dc:128 * dc + 128, :]
        for tap in range(3) for dc in range(4)
    ]) * 16.0
    w1_a = img([
        np.ascontiguousarray(w1.T)[128 * dc:128 * dc + 128, 128 * fc:128 * fc + 128]
        for dc in range(4) for fc in range(8)
    ])
    w2t_a = img([
        np.ascontiguousarray(w2.T)[128 * fc:128 * fc + 128, :]
        for fc in range(8)
    ])

    r = np.arange(128)
    m_lo = (r[:, None] >= r[None, :]).astype(np.float32)   # block 0: keep k>=q
    mask01 = np.concatenate([m_lo, m_lo.T], axis=1)

    ident = np.eye(128, dtype=np.float32)

    def bf(a):
        import ml_dtypes
        return np.asarray(a, dtype=ml_dtypes.bfloat16)

    def f8(a):
        import ml_dtypes
        return np.asarray(a, dtype=ml_dtypes.float8_e4m3fn)

    common = {
        "wk": f8(wk_a * 16.0), "wq": f8(wq_a * 16.0), "wu": f8(wu_a), "wo": bf(wo_a),
        "w1": bf(w1_a), "w2t": bf(w2t_a), "mask01": bf(mask01),
        "ident": bf(ident),
    }

    in_maps = []
    for c in range(N_CORES):
        b, j = divmod(c, 4)
        s = 512 * j
        xe = np.zeros((XE, D), np.float32)
        lo, hi = max(0, s - 257), min(T, s + 769)
        xe[lo - (s - 257): hi - (s - 257)] = x[b, lo:hi]
        xt = xe.T.reshape(4, 128, XE).transpose(1, 0, 2).reshape(128, 4 * XE)
        xt = np.ascontiguousarray(xt)
        XE8 = 1040
        xt8 = np.zeros((128, 4, XE8), np.float32)
        xt8[:, :, :XE] = xt.reshape(128, 4, XE)
        xt8 = xt8.reshape(128, 4 * XE8)

        # vones[p, t] = 1 iff kv token (s - 256 + 128 t + p) is in-sequence;
        # replicated x64 it fills the ones-columns of v, so the softmax
        # denominator (PSUM rows 64-127) skips zero-padded kv exactly.
        kvidx = s - 256 + 128 * np.arange(8)[None, :] + r[:, None]
        vones = ((kvidx >= 0) & (kvidx < T)).astype(np.float32)
        vones = np.repeat(vones, 64, axis=1)

        m = dict(common)
        m["xt"] = bf(xt)
        m["xt8"] = f8(xt8)
        m["vones"] = bf(vones)
        in_maps.append(m)
    return in_maps


def kernel(**inputs) -> np.ndarray:
    if "nc" not in _cached:
        _cached["nc"] = _build_program()
    nc = _cached["nc"]
    in_maps = _prep_host(inputs)
    res = run_bass_kernel_spmd(nc, in_maps, core_ids=list(range(N_CORES)))
    out = np.empty((B, T, D), np.float32)
    for c in range(N_CORES):
        b, j = divmod(c, 4)
        o = np.asarray(res.results[c]["out"], np.float32)
        o = o.reshape(128, 4, 512).transpose(1, 0, 2)
        out[b, 512 * j: 512 * j + 512] = o.reshape(512, 512)
    return out


# revision 67
# speedup vs baseline: 1.2514x; 1.2514x over previous
"""Trainium2 Bass kernel for nn_MicroExpert (sparse_attention).

Reference model (B=2, T=2048, D=512, H=8, HD=64):
  v_in = conv1d(x, k=3, pad=1); MHA(q=x, k=x, v=v_in) with banded mask
  |i-j| <= 256; h = LN(x + attn); out = LN(h + FFN(h)).

Sharding: data-parallel over (batch, 512-token chunk) -> 8 independent
cores, no collectives.  Each core recomputes the K/V halo (+-256 tokens,
zero-padded at sequence edges); pad keys drop out of the softmax exactly
because the valid-kv indicator (`vones`) forms the ones-columns of v.

Key structure:
- K/Q/V projections run in fp8 (e4m3, weights x16-prescaled) with
  DoubleRow so each matmul contracts 256 rows; the conv is folded into
  the V projection on the host: v[t] = sum_d U_d x[t+d-1],
  U_d = Wv conv_w[:,:,d].
- Attention keeps a [feature, token] layout.  Scores for a head pair
  are issued into one PSUM tile with alternating 64-row groups (both
  K=64 matmuls run concurrently) and exponentiated by a single Exp; the
  scalar engine runs only Exp during attention so its activation table
  never swaps.  The v stationary carries 64 replicated valid-kv ones
  columns, so the ctx matmul emits the softmax denominator on PSUM rows
  64-127, partition-aligned with ctx - normalization is one reciprocal
  plus two multiplies, no transpose/broadcast.  The qt loop is software-
  pipelined one iteration deep to hide the denominator latency.
- LayerNorm uses bn_stats/bn_aggr; LN1 finishes after the qt loop.
- FFN2 uses m1 as the stationary operand so its output lands directly
  in [token, feature]; residual2 + LN2 + store pipeline per query tile.
"""

import os
import sys

import numpy as np

sys.path.insert(0, "/opt/trn_rl_repo")

import concourse.bass as bass
import concourse.mybir as mybir
import concourse.tile as tile
from concourse import bacc
from concourse.bass_utils import run_bass_kernel_spmd

BF16 = mybir.dt.bfloat16
F32 = mybir.dt.float32

B, T, D, H, HD = 2, 2048, 512, 8, 64
S = 512          # tokens per core
KV = 1024        # extended kv tokens per core (S + 2*256)
XE = 1026        # x_ext width (KV + 2 for conv halo)
NQT = 4          # 128-query tiles per core
NKT = 5          # relative 128-key tiles per query tile
F = 1024         # FFN hidden
EPS = 1e-5
N_CORES = 8

_cached = {}


def _build_program():
    nc = bacc.Bacc("TRN2", target_bir_lowering=False, debug=False)

    # ---- DRAM tensors -------------------------------------------------
    def din(name, shape, dt):
        return nc.dram_tensor(name, shape, dt, kind="ExternalInput").ap()

    F8 = mybir.dt.float8e4
    XE8 = 1040  # fp8 xt block stride (DoubleRow needs step % 16 == 0)

    # all inputs are pre-layouted [128, N] SBUF images (host does the packing)
    xt_d = din("xt", [128, 4 * XE], BF16)
    xt8_d = din("xt8", [128, 4 * XE8], F8)
    wk_d = din("wk", [128, 2048], F8)                 # x16 scaled, fp8
    wq_d = din("wq", [128, 2048], F8)                 # x16 scaled, fp8
    wu_d = din("wu", [128, 6144], F8)                 # x16 scaled, fp8
    wo_d = din("wo", [128, 2048], BF16)
    w1_d = din("w1", [128, 4096], F8)                 # x16 scaled, fp8
    w2t_d = din("w2t", [128, 4096], F8)               # x16 scaled, fp8
    mask_d = din("mask01", [128, 256], BF16)          # [tril | triu] 0/1
    vones_d = din("vones", [128, 512], BF16)          # valid-kv, x64 replicated
    ident_d = din("ident", [128, 128], BF16)

    out_d = nc.dram_tensor("out", [128, 2048], BF16, kind="ExternalOutput").ap()

    with tile.TileContext(nc) as tc:
        from contextlib import ExitStack

        with ExitStack() as ctx:
            const = ctx.enter_context(tc.tile_pool(name="const", bufs=1))

            # ---- load constants/weights (consumption order, parallel queues)
            def load_w(dram, cols, dt=BF16, parts=128, nchunk=1, eng=None):
                t = const.tile([parts, cols], dt, name=f"w_{dram.tensor.name}")
                step = cols // nchunk
                e = eng if eng is not None else nc.sync
                for c0 in range(0, cols, step):
                    e.dma_start(t[:, c0:c0 + step], dram[:, c0:c0 + step])
                return t

            # single queue, strict consumption order (priority = order)
            wk_sb = load_w(wk_d, 2048, dt=F8)
            xt8_sb = load_w(xt8_d, 4 * XE8, dt=F8, nchunk=4)
            wq_sb = load_w(wq_d, 2048, dt=F8)
            xt_sb = load_w(xt_d, 4 * XE, nchunk=2)
            wu_sb = load_w(wu_d, 6144, dt=F8)
            wo_sb = load_w(wo_d, 2048)
            w1_sb = load_w(w1_d, 4096, dt=F8)
            w2t_sb = load_w(w2t_d, 4096, dt=F8)
            mask_sb = load_w(mask_d, 256)
            vones_sb = load_w(vones_d, 512)
            ident_sb = load_w(ident_d, 128)

            # persistent activations
            kt_sb = const.tile([128, 4 * KV], BF16)    # [oc-block][kv]
            q_sb = const.tile([128, 4 * S], BF16)      # [oc-block][tok]
            v_sb = const.tile([128, 8 * 1024], BF16)   # [kvtile][(v_h | ones x64) x 8]
            ctx_sb = const.tile([128, 4 * 512], BF16)  # [qt-block][pair-block][q]
            hn_sb = const.tile([128, 4 * 512], BF16)   # [qt-block][D]  (LN1 out, [tok,D])
            h1t_sb = const.tile([128, 4 * 512], F8)    # [dc-block][tok], fp8
            m1_sb = const.tile([128, 8 * 512], F8)     # [fc-block][tok], fp8
            out_sb = const.tile([128, 4 * 512], BF16)  # [qt-block][D]
            stats = const.tile([128, 32], F32)
            eps_sb = const.tile([128, 1], F32)
            nc.gpsimd.memset(eps_sb[:], float(EPS))

            xt_v = xt_sb[:].rearrange("p (c w) -> p c w", c=4)
            # valid-kv indicator, replicated across 64 stationary columns of
            # each head's v slice: the ctx matmul then emits the softmax
            # denominator (pad-kv excluded) on PSUM rows 64-127, partition-
            # aligned with the ctx rows, so normalization needs no
            # transpose/broadcast of the denominator at all.
            v_thw = v_sb[:].rearrange("p (t h w) -> p t h w", t=8, h=8)
            for h in range(8):
                nc.gpsimd.tensor_copy(
                    v_thw[:, :, h, 64:128],
                    vones_sb[:].rearrange("p (t o) -> p t o", t=8),
                )

            # ---- projections: kT, qT, v (fp8 DoubleRow) ----------------
            with tc.tile_pool(name="pp", bufs=2, space="PSUM") as pp_pool:
                xt8_v = xt8_sb[:].rearrange("p (c w) -> p c w", c=4)
                # [p, j(kc-pair), k(kc-in-pair), oc*128]
                wkv = wk_sb[:].rearrange("p (j k w) -> p j k w", j=2, k=2)
                wqv = wq_sb[:].rearrange("p (j k w) -> p j k w", j=2, k=2)
                for oc in range(4):
                    for half in range(2):
                        pp = pp_pool.tile([128, 512], F32, tag="pp")
                        for j in range(2):
                            nc.tensor.matmul(
                                pp[:],
                                wkv[:, j, :, 128 * oc: 128 * oc + 128],
                                xt8_v[:, 2 * j:2 * j + 2,
                                      1 + 512 * half: 1 + 512 * half + 512],
                                start=(j == 0), stop=(j == 1),
                                perf_mode=mybir.MatmulPerfMode.DoubleRow,
                            )
                        dst = kt_sb[:, KV * oc + 512 * half: KV * oc + 512 * half + 512]
                        if (oc + half) % 2 == 0:
                            nc.scalar.copy(dst, pp[:])
                        else:
                            nc.vector.tensor_copy(dst, pp[:])
                for oc in range(4):
                    pp = pp_pool.tile([128, 512], F32, tag="pp")
                    for j in range(2):
                        nc.tensor.matmul(
                            pp[:],
                            wqv[:, j, :, 128 * oc: 128 * oc + 128],
                            xt8_v[:, 2 * j:2 * j + 2, 257: 257 + 512],
                            start=(j == 0), stop=(j == 1),
                            perf_mode=mybir.MatmulPerfMode.DoubleRow,
                        )
                    nc.vector.tensor_copy(q_sb[:, 512 * oc: 512 * oc + 512], pp[:])
                xt8_v = xt8_sb[:].rearrange("p (c w) -> p c w", c=4)
                wu_v = wu_sb[:].rearrange("p (j b w) -> p j b w", j=6, b=2)
                for tt in range(8):
                    pp = pp_pool.tile([128, 512], F32, tag="pp")
                    for j in range(6):  # DoubleRow: two (tap,dc) blocks per MM
                        tap, k2 = j // 2, 2 * (j % 2)
                        nc.tensor.matmul(
                            pp[:],
                            xt8_v[:, k2:k2 + 2, 128 * tt + tap: 128 * tt + tap + 128],
                            wu_v[:, j],
                            start=(j == 0), stop=(j == 5),
                            perf_mode=mybir.MatmulPerfMode.DoubleRow,
                        )
                    vv = v_sb[:, 1024 * tt: 1024 * tt + 1024].rearrange(
                        "p (h w) -> p h w", h=8
                    )
                    src = pp[:].rearrange("p (h w) -> p h w", h=8)
                    if tt % 2 == 0:
                        nc.scalar.activation(
                            vv[:, :, 0:64], src,
                            mybir.ActivationFunctionType.Copy, scale=1.0 / 16,
                        )
                    else:
                        nc.vector.tensor_scalar_mul(vv[:, :, 0:64], src, 1.0 / 16)

            # ---- attention (software-pipelined over qt) ----------------
            with ExitStack() as actx:
                sc_pool = actx.enter_context(tc.tile_pool(name="scps", bufs=2, space="PSUM"))
                cx_pool = actx.enter_context(tc.tile_pool(name="cxps", bufs=1, space="PSUM"))
                ex_pool = actx.enter_context(tc.tile_pool(name="expsb", bufs=4))
                dn_pool = actx.enter_context(tc.tile_pool(name="densb", bufs=2))
                r1t_pool = actx.enter_context(tc.tile_pool(name="r1t", bufs=2))
                r1_pool = actx.enter_context(tc.tile_pool(name="r1", bufs=4))

                cxps_t = [None] * NQT

                def sc_stage(qt, p):
                    """Scores + exp + mask for head pair p (heads 2p, 2p+1):
                    one PSUM tile and a single Exp for both halves."""
                    scps = sc_pool.tile([128, 1280], F32, tag="sc")
                    # interleave halves so row-groups 0-63 / 64-127 overlap on PE
                    for kt in range(NKT):
                        for half in range(2):
                            hp = 64 * half
                            kcol = KV * p + 128 * (qt + kt)
                            nc.tensor.matmul(
                                scps[:, 640 * half + 128 * kt: 640 * half + 128 * kt + 128],
                                kt_sb[hp:hp + 64, kcol:kcol + 128],
                                q_sb[hp:hp + 64, 512 * p + 128 * qt: 512 * p + 128 * qt + 128],
                                start=True, stop=True,
                            )
                    expair = ex_pool.tile([128, 1280], BF16, tag="ex")
                    nc.scalar.activation(
                        expair[:], scps[:], mybir.ActivationFunctionType.Exp,
                        scale=float(1.0 / (np.sqrt(HD) * 256.0)),
                    )
                    exs = []
                    for half in range(2):
                        ex = expair[:, 640 * half: 640 * half + 640]
                        # band mask on relative tiles 0 and 4 (multiplicative 0/1)
                        ex_edge = ex.rearrange("p (a b) -> p a b", a=5)[:, ::4, :]
                        nc.vector.tensor_mul(
                            ex_edge, ex_edge,
                            mask_sb[:].rearrange("p (n w) -> p n w", n=2),
                        )
                        exs.append(ex)
                    return exs

                def ctx_stage(qt, p, exs):
                    if cxps_t[qt] is None:
                        cxps = cx_pool.tile([128, 1024], F32, tag="cx", name=f"cx{qt}")
                        cxps_t[qt] = cxps
                    cxps = cxps_t[qt]
                    for half in range(2):
                        h = 2 * p + half
                        ex = exs[half]
                        for kt in range(NKT):
                            nc.tensor.matmul(
                                cxps[:, 128 * h: 128 * h + 128],
                                v_sb[:, 1024 * (qt + kt) + 128 * h: 1024 * (qt + kt) + 128 * h + 128],
                                ex[:, 128 * kt: 128 * kt + 128],
                                start=(kt == 0), stop=(kt == NKT - 1),
                                skip_group_check=True,
                            )

                def den_stage(qt):
                    """Copy ctx and replicated denominator out of PSUM; take
                    1/den at partition base 0 (reciprocal_approx_* misreads
                    PSUM and non-zero partition bases on hardware)."""
                    cxps = cxps_t[qt]
                    ctxu = dn_pool.tile([64, 1024], BF16, tag="ctxu")
                    nc.vector.tensor_copy(ctxu[:], cxps[0:64, :])
                    den = dn_pool.tile([64, 1024], F32, tag="den")
                    nc.vector.tensor_copy(den[:], cxps[64:128, :])
                    rden = dn_pool.tile([64, 1024], F32, tag="rden")
                    nc.vector.reciprocal_approx_fast(rden[:], den[:])
                    cxps_t[qt] = None
                    return ctxu, rden

                def bcast_norm(qt, ctxu, rden):
                    cxv = ctxu[:].rearrange("p (a b) -> p a b", a=8)
                    bcv = rden[:].rearrange("p (a b) -> p a b", a=8)
                    ctxv = ctx_sb[:].rearrange("p (a b) -> p a b", a=16)
                    for par in range(2):
                        nc.vector.tensor_mul(
                            ctxv[64 * par:64 * par + 64, 4 * qt:4 * qt + 4, :],
                            cxv[:, par::2, :],
                            bcv[:, par::2, :],
                        )

                r1_tiles = []

                def oproj_r1(qt):
                    """out_proj + residual + transpose to [tok, feat]."""
                    atps = sc_pool.tile([128, 512], F32, tag="sc")
                    for oc in range(4):
                        for pc in range(4):
                            nc.tensor.matmul(
                                atps[:, 128 * oc: 128 * oc + 128],
                                wo_sb[:, 128 * (4 * pc + oc): 128 * (4 * pc + oc) + 128],
                                ctx_sb[:, 512 * qt + 128 * pc: 512 * qt + 128 * pc + 128],
                                start=(pc == 0), stop=(pc == 3),
                            )
                    r1t = r1t_pool.tile([128, 512], BF16, tag="r1t")
                    nc.vector.tensor_add(
                        r1t[:], atps[:], xt_v[:, :, 257 + 128 * qt: 257 + 128 * qt + 128]
                    )
                    tpps = sc_pool.tile([128, 512], BF16, tag="sc")
                    for dc in range(4):
                        nc.tensor.transpose(
                            tpps[:, 128 * dc: 128 * dc + 128],
                            r1t[:, 128 * dc: 128 * dc + 128],
                            ident_sb[:],
                        )
                    r1 = r1_pool.tile([128, 512], BF16, tag="r1", name=f"r1_{qt}")
                    nc.vector.tensor_copy(r1[:], tpps[:])
                    r1_tiles.append(r1)
                    b6 = dn_pool.tile([128, 6], F32, tag="b6")
                    nc.vector.bn_stats(b6[:], r1[:])
                    nc.vector.bn_aggr(stats[:, 2 * qt:2 * qt + 2], b6[:])

                # software pipeline with one-iteration lookahead: qt's
                # scores/ctx cover qt-1's denominator-chain latency, then
                # qt-1's normalize/out_proj fill PE behind them.  LayerNorm1
                # is deferred past the loop so the scalar engine runs Exp
                # only (no activation-table swaps) and no PE op waits on
                # the LN chain.
                pend = None
                for qt in range(NQT):
                    ex0 = sc_stage(qt, 0)
                    ex1 = sc_stage(qt, 1)
                    ctx_stage(qt, 0, ex0)
                    ex2 = sc_stage(qt, 2)
                    ctx_stage(qt, 1, ex1)
                    ex3 = sc_stage(qt, 3)
                    ctx_stage(qt, 2, ex2)
                    if pend is not None:
                        bcast_norm(qt - 1, *pend)
                    ctx_stage(qt, 3, ex3)
                    if pend is not None:
                        oproj_r1(qt - 1)
                    pend = den_stage(qt)
                def ln1_finish(qt):
                    nc.scalar.activation(
                        stats[:, 16 + qt:17 + qt], stats[:, 2 * qt + 1:2 * qt + 2],
                        mybir.ActivationFunctionType.Sqrt, bias=eps_sb[:, 0:1],
                    )
                    nc.vector.reciprocal(
                        stats[:, 20 + qt:21 + qt], stats[:, 16 + qt:17 + qt]
                    )
                    nc.vector.tensor_scalar(
                        hn_sb[:, 512 * qt: 512 * qt + 512], r1_tiles[qt][:],
                        stats[:, 2 * qt:2 * qt + 1], stats[:, 20 + qt:21 + qt],
                        op0=mybir.AluOpType.subtract, op1=mybir.AluOpType.mult,
                    )
                    tpps2 = sc_pool.tile([128, 512], BF16, tag="sc")
                    for dc in range(4):
                        nc.tensor.transpose(
                            tpps2[:, 128 * dc: 128 * dc + 128],
                            hn_sb[:, 512 * qt + 128 * dc: 512 * qt + 128 * dc + 128],
                            ident_sb[:],
                        )
                    h1tv = h1t_sb[:].rearrange("p (c w) -> p c w", c=4)
                    nc.vector.tensor_copy(
                        h1tv[:, :, 128 * qt: 128 * qt + 128], tpps2[:]
                    )

                bcast_norm(NQT - 1, *pend)
                oproj_r1(NQT - 1)
                for qt in range(NQT):
                    ln1_finish(qt)

            # ---- FFN1 --------------------------------------------------
            with tc.tile_pool(name="fps", bufs=2, space="PSUM") as f_pool:
                w1v = w1_sb[:].rearrange("p (j k w) -> p j k w", j=2, k=2)
                h1t8_v = h1t_sb[:].rearrange("p (c w) -> p c w", c=4)
                for fc in range(8):
                    fps = f_pool.tile([128, 512], F32, tag="f")
                    for j in range(2):
                        nc.tensor.matmul(
                            fps[:],
                            w1v[:, j, :, 128 * fc: 128 * fc + 128],
                            h1t8_v[:, 2 * j:2 * j + 2, :],
                            start=(j == 0), stop=(j == 1),
                            perf_mode=mybir.MatmulPerfMode.DoubleRow,
                        )
                    nc.vector.tensor_scalar(
                        m1_sb[:, 512 * fc: 512 * fc + 512], fps[:],
                        1.0 / 16, 0.0,
                        op0=mybir.AluOpType.mult, op1=mybir.AluOpType.max,
                    )

                # ---- FFN2 ([tok, feat] direct) + residual2 + LN2 + store
                with ExitStack() as fctx:
                    r2_pool = fctx.enter_context(tc.tile_pool(name="r2", bufs=2))
                    sq2_pool = fctx.enter_context(tc.tile_pool(name="sq2", bufs=2))
                    m1v = m1_sb[:].rearrange("p (c w) -> p c w", c=8)
                    w2tv = w2t_sb[:].rearrange("p (j k w) -> p j k w", j=4, k=2)
                    for qt in range(NQT):
                        fps = f_pool.tile([128, 512], F32, tag="f")
                        for j in range(4):
                            nc.tensor.matmul(
                                fps[:],
                                m1v[:, 2 * j:2 * j + 2, 128 * qt: 128 * qt + 128],
                                w2tv[:, j],
                                start=(j == 0), stop=(j == 3),
                                perf_mode=mybir.MatmulPerfMode.DoubleRow,
                            )
                        r2 = r2_pool.tile([128, 512], BF16, tag="r2")
                        nc.vector.scalar_tensor_tensor(
                            r2[:], fps[:], 1.0 / 16,
                            hn_sb[:, 512 * qt: 512 * qt + 512],
                            op0=mybir.AluOpType.mult, op1=mybir.AluOpType.add,
                        )
                        b6 = sq2_pool.tile([128, 6], F32, tag="b6f")
                        mv = stats[:, 8 + 2 * qt:10 + 2 * qt]
                        nc.vector.bn_stats(b6[:], r2[:])
                        nc.vector.bn_aggr(mv, b6[:])
                        sd = stats[:, 24 + qt:25 + qt]
                        nc.scalar.activation(
                            sd, stats[:, 9 + 2 * qt:10 + 2 * qt],
                            mybir.ActivationFunctionType.Sqrt, bias=eps_sb[:, 0:1],
                        )
                        rstd = stats[:, 28 + qt:29 + qt]
                        nc.vector.reciprocal(rstd, sd)
                        nc.vector.tensor_scalar(
                            out_sb[:, 512 * qt: 512 * qt + 512], r2[:],
                            stats[:, 8 + 2 * qt:9 + 2 * qt], rstd,
                            op0=mybir.AluOpType.subtract, op1=mybir.AluOpType.mult,
                        )
                        nc.sync.dma_start(
                            out_d[:, 512 * qt: 512 * qt + 512],
                            out_sb[:, 512 * qt: 512 * qt + 512],
                        )

    nc.compile()
    return nc


def _prep_host(inputs):
    x = np.asarray(inputs["x"], np.float32)
    conv_w = np.asarray(inputs["conv_w"], np.float32)
    conv_b = np.asarray(inputs["conv_b"], np.float32)
    in_w = np.asarray(inputs["in_proj_w"], np.float32)
    in_b = np.asarray(inputs["in_proj_b"], np.float32)
    out_w = np.asarray(inputs["out_proj_w"], np.float32)
    out_b = np.asarray(inputs["out_proj_b"], np.float32)
    w1 = np.asarray(inputs["w1"], np.float32)
    b1 = np.asarray(inputs["b1"], np.float32)
    w2 = np.asarray(inputs["w2"], np.float32)
    b2 = np.asarray(inputs["b2"], np.float32)
    g1 = np.asarray(inputs["ln1_g"], np.float32)
    bb1 = np.asarray(inputs["ln1_b"], np.float32)
    g2 = np.asarray(inputs["ln2_g"], np.float32)
    bb2 = np.asarray(inputs["ln2_b"], np.float32)

    for nm, v in (("conv_b", conv_b), ("in_proj_b", in_b), ("out_proj_b", out_b),
                  ("b1", b1), ("b2", b2)):
        if np.any(v != 0):
            raise NotImplementedError(f"nonzero {nm} unsupported")
    if np.any(g1 != 1) or np.any(bb1 != 0) or np.any(g2 != 1) or np.any(bb2 != 0):
        raise NotImplementedError("nontrivial layernorm affine unsupported")

    Wq, Wk, Wv = in_w[:D], in_w[D:2 * D], in_w[2 * D:]
    U = [(Wv @ conv_w[:, :, d]) for d in range(3)]  # v[t] = sum U_d @ x[t+d-1]

    def img(stack):  # [n, 128, w] slices -> [128, n*w] SBUF image
        a = np.asarray(stack, np.float32)
        return np.ascontiguousarray(a.transpose(1, 0, 2).reshape(128, -1))

    def slc16(W):  # W used as out = W @ x  -> lhsT slices of W.T
        WT = np.ascontiguousarray(W.T)
        return img([
            WT[128 * kc:128 * kc + 128, 128 * oc:128 * oc + 128]
            for kc in range(4) for oc in range(4)
        ])

    wk_a = slc16(Wk)
    wq_a = slc16(Wq)
    wo_a = slc16(out_w)
    wu_a = img([
        np.ascontiguousarray(U[tap].T)[128 * dc:128 * dc + 128, :]
        for tap in range(3) for dc in range(4)
    ]) * 16.0
    w1_a = img([
        np.ascontiguousarray(w1.T)[128 * dc:128 * dc + 128, 128 * fc:128 * fc + 128]
        for dc in range(4) for fc in range(8)
    ])
    w2t_a = img([
        np.ascontiguousarray(w2.T)[128 * fc:128 * fc + 128, :]
        for fc in range(8)
    ])

    r = np.arange(128)
    m_lo = (r[:, None] >= r[None, :]).astype(np.float32)   # block 0: keep k>=q
    mask01 = np.concatenate([m_lo, m_lo.T], axis=1)

    ident = np.eye(128, dtype=np.float32)

    def bf(a):
        import ml_dtypes
        return np.asarray(a, dtype=ml_dtypes.bfloat16)

    def f8(a):
        import ml_dtypes
        return np.asarray(a, dtype=ml_dtypes.float8_e4m3fn)

    common = {
        "wk": f8(wk_a * 16.0), "wq": f8(wq_a * 16.0), "wu": f8(wu_a), "wo": bf(wo_a),
        "w1": f8(w1_a * 16.0), "w2t": f8(w2t_a * 16.0), "mask01": bf(mask01),
        "ident": bf(ident),
    }

    in_maps = []
    for c in range(N_CORES):
        b, j = divmod(c, 4)
        s = 512 * j
        xe = np.zeros((XE, D), np.float32)
        lo, hi = max(0, s - 257), min(T, s + 769)
        xe[lo - (s - 257): hi - (s - 257)] = x[b, lo:hi]
        xt = xe.T.reshape(4, 128, XE).transpose(1, 0, 2).reshape(128, 4 * XE)
        xt = np.ascontiguousarray(xt)
        XE8 = 1040
        xt8 = np.zeros((128, 4, XE8), np.float32)
        xt8[:, :, :XE] = xt.reshape(128, 4, XE)
        xt8 = xt8.reshape(128, 4 * XE8)

        # vones[p, t] = 1 iff kv token (s - 256 + 128 t + p) is in-sequence;
        # replicated x64 it fills the ones-columns of v, so the softmax
        # denominator (PSUM rows 64-127) skips zero-padded kv exactly.
        kvidx = s - 256 + 128 * np.arange(8)[None, :] + r[:, None]
        vones = ((kvidx >= 0) & (kvidx < T)).astype(np.float32)
        vones = np.repeat(vones, 64, axis=1)

        m = dict(common)
        m["xt"] = bf(xt)
        m["xt8"] = f8(xt8)
        m["vones"] = bf(vones)
        in_maps.append(m)
    return in_maps


def kernel(**inputs) -> np.ndarray:
    if "nc" not in _cached:
        _cached["nc"] = _build_program()
    nc = _cached["nc"]
    in_maps = _prep_host(inputs)
    res = run_bass_kernel_spmd(nc, in_maps, core_ids=list(range(N_CORES)))
    out = np.empty((B, T, D), np.float32)
    for c in range(N_CORES):
        b, j = divmod(c, 4)
        o = np.asarray(res.results[c]["out"], np.float32)
        o = o.reshape(128, 4, 512).transpose(1, 0, 2)
        out[b, 512 * j: 512 * j + 512] = o.reshape(512, 512)
    return out
